# revision 15
# baseline (speedup 1.0000x reference)
"""Trainium2 Bass kernel for nn_ContrastiveLoss (N=4096, D=1024).

Strategy (8 NeuronCores, 2 row-groups x 4 col-groups):
  Core (r, g) computes the [2048 x 1024] block of both exp-cosine
  similarity matrices Sxx = exp(cos(x_i, x_j)/T) and Sxy =
  exp(cos(x_i, y_j)/T) for rows r*2048..(r+1)*2048, cols
  g*1024..(g+1)*1024, reducing each row on the fly (ScalarE fused
  exp+row-accumulate).  Matmuls run as fp32r on TensorE with the
  contraction (feature) dim on partitions; the column operands are
  pre-normalized (column 1/norm broadcast built with a rank-1 matmul),
  the row 1/norm is folded into the ScalarE exp scale.  Each core also
  owns a disjoint 512-row slice for the positive-pair/JS-divergence
  per-row terms.  The host only assembles per-core partial row sums,
  does the O(N) cumsum/log (the sequential cross-core prefix), and the
  final scalar reduction.
"""

import numpy as np

T = 0.15
N, D = 4096, 1024
NCORES = 8
CGRP = 4               # col-groups
RB = N // 2            # row-block rows per core (2 row groups)
CB = N // CGRP         # col-block cols per core
JB = N // NCORES       # js/positive-pair rows per core
FREE = 512             # matmul moving free size
P = 128


def build(nc, tc, io, d=D, rb=RB, cb=CB, jb=JB, free=FREE):
    """Emit the per-core Tile program.  ``io`` maps tensor name -> AP."""
    import concourse.mybir as mybir
    from concourse.alu_op_type import AluOpType
    from bass_rust import AxisListType as AX

    f32 = mybir.dt.float32
    f32r = mybir.dt.float32r
    AF = mybir.ActivationFunctionType
    nch = d // P          # feature chunks
    nt = rb // P          # row tiles
    ng = cb // free       # col groups per matrix
    nj = jb // P          # js tiles
    ncol = 2 * ng         # accum columns per row tile (2 matrices)

    def ap(name):
        return io[name]

    xrT, xcT, ycT = ap("xrT"), ap("xcT"), ap("ycT")
    xrj, xj, yj = ap("xrj"), ap("xj"), ap("yj")

    with (
        tc.tile_pool(name="big", bufs=1) as big,
        tc.tile_pool(name="sq", bufs=2) as sqp,
        tc.tile_pool(name="xrjp", bufs=2) as xrjp,
        tc.tile_pool(name="expp", bufs=3) as expp,
        tc.tile_pool(name="jsin", bufs=1) as jsin,
        tc.tile_pool(name="jse", bufs=1) as jse,
        tc.tile_pool(name="jstmp", bufs=4) as jstmp,
        tc.tile_pool(name="small", bufs=1) as small,
        tc.tile_pool(name="tiny", bufs=2) as tiny,
        tc.tile_pool(name="mpsum", bufs=6, space="PSUM") as mpsum,
        tc.tile_pool(name="npsum", bufs=2, space="PSUM") as npsum,
        # NOTE: every distinct tag gets its own `bufs` slots -- keep one
        # tag per pool.
    ):
        # ---- persistent SBUF tensors ----
        xr_sb = big.tile([P, nch * rb], f32r)      # feature-major row operand
        xc_sb = big.tile([P, nch * cb], f32r)      # feature-major col operands
        yc_sb = big.tile([P, nch * cb], f32r)
        bc_x = big.tile([P, cb], f32)              # col 1/norm broadcast tiles
        bc_y = big.tile([P, cb], f32)
        ones_col = small.tile([P, 1], f32r)
        ones_row = small.tile([1, P], f32r)
        ss_cols = small.tile([1, 2 * cb], f32r)     # col sumsq -> 1/norm

        ssx_sb = small.tile([P, nt], f32)          # row-block row sumsq
        nrm_r = small.tile([P, nt], f32)
        invr_T = small.tile([P, nt], f32)          # (1/norm_row)/T
        rs_acc = small.tile([P, nt * ncol], f32)   # fused exp row sums
        ssy_sb = small.tile([P, nj], f32)
        dot_sb = small.tile([P, nj], f32)
        sx_sb = small.tile([P, nj], f32)
        sy_sb = small.tile([P, nj], f32)
        exs_sb = small.tile([P, nj], f32)
        eys_sb = small.tile([P, nj], f32)
        w_sb = small.tile([P, nj], f32)

        ones_f32 = small.tile([P, 1], f32)
        onesr_f32 = small.tile([1, P], f32)
        nc.vector.memset(ones_f32[:], 1.0)
        nc.vector.memset(onesr_f32[:], 1.0)
        nc.vector.tensor_copy(ones_col[:], ones_f32[:])
        nc.vector.tensor_copy(ones_row[:], onesr_f32[:])

        # ---- loads ----
        for ch in range(nch):
            nc.sync.dma_start(
                xc_sb[:, ch * cb:(ch + 1) * cb], xcT[ch * P:(ch + 1) * P, :])
        xrj_tiles = []
        for t in range(nt):
            xt = xrjp.tile([P, d], f32)
            nc.sync.dma_start(xt[:], xrj[t * P:(t + 1) * P, :])
            xrj_tiles.append(xt)
        for ch in range(nch):
            nc.sync.dma_start(
                xr_sb[:, ch * rb:(ch + 1) * rb], xrT[ch * P:(ch + 1) * P, :])
        for ch in range(nch):
            nc.sync.dma_start(
                yc_sb[:, ch * cb:(ch + 1) * cb], ycT[ch * P:(ch + 1) * P, :])

        # ---- row sumsq from row-major tiles (ScalarE square + accum) ----
        for t in range(nt):
            nc.scalar.activation(xrj_tiles[t][:], xrj_tiles[t][:], AF.Square,
                                 accum_out=ssx_sb[:, t:t + 1])
        # 1/norm_row / T
        nc.scalar.activation(nrm_r[:], ssx_sb[:], AF.Sqrt)
        nc.vector.reciprocal(invr_T[:], nrm_r[:])
        nc.vector.tensor_scalar_mul(invr_T[:], invr_T[:], 1.0 / T)

        # ---- col sumsq: square then ones-matmul partition reduce ----
        for idx, (src, base) in enumerate(((xc_sb, 0), (yc_sb, cb))):
            for g in range(cb // free):
                ps = npsum.tile([1, free], f32, tag="np")
                for ch in range(nch):
                    sq = sqp.tile([P, free], f32r, tag="sqcol")
                    nc.scalar.activation(
                        sq[:], src[:, ch * cb + g * free: ch * cb + (g + 1) * free],
                        AF.Square)
                    nc.tensor.matmul(ps[:], ones_col[:], sq[:],
                                     start=(ch == 0), stop=(ch == nch - 1))
                nc.vector.tensor_copy(
                    ss_cols[0:1, base + g * free: base + (g + 1) * free], ps[:])
        nc.scalar.activation(ss_cols[:], ss_cols[:], AF.Sqrt)
        with nc.allow_low_precision(reason="fp32r rounding of 1/norm feeds "
                                    "the fp32r broadcast matmul"):
            nc.vector.reciprocal(ss_cols[:], ss_cols[:])

        # ---- broadcast col 1/norm across partitions (rank-1 matmul) ----
        for idx, dst in enumerate((bc_x, bc_y)):
            for g in range(cb // free):
                ps = mpsum.tile([P, free], f32, tag="mm")
                nc.tensor.matmul(
                    ps[:], ones_row[:],
                    ss_cols[0:1, idx * cb + g * free: idx * cb + (g + 1) * free],
                    start=True, stop=True)
                nc.vector.tensor_copy(dst[:, g * free:(g + 1) * free], ps[:])

        # ---- normalize column operands ----
        for ch in range(nch):
            nc.vector.tensor_mul(xc_sb[:, ch * cb:(ch + 1) * cb],
                                 xc_sb[:, ch * cb:(ch + 1) * cb], bc_x[:])
        for ch in range(nch):
            nc.vector.tensor_mul(yc_sb[:, ch * cb:(ch + 1) * cb],
                                 yc_sb[:, ch * cb:(ch + 1) * cb], bc_y[:])

        # ---- js block emitter ----
        def emit_js(j):
            xt = jsin.tile([P, d], f32, tag="jsx")
            nc.sync.dma_start(xt[:], xj[j * P:(j + 1) * P, :])
            yt = jsin.tile([P, d], f32, tag="jsy")
            nc.sync.dma_start(yt[:], yj[j * P:(j + 1) * P, :])
            sq = jstmp.tile([P, d], f32, tag="jt", name=f"sq_{j}")
            nc.scalar.activation(sq[:], yt[:], AF.Square,
                                 accum_out=ssy_sb[:, j:j + 1])
            prod = jstmp.tile([P, d], f32, tag="jt", name=f"prod_{j}")
            nc.vector.tensor_mul(prod[:], xt[:], yt[:])
            nc.vector.reduce_sum(dot_sb[:, j:j + 1], prod[:], axis=AX.X)
            ex = jse.tile([P, d], f32, tag="ex")
            nc.scalar.activation(ex[:], xt[:], AF.Exp,
                                 accum_out=sx_sb[:, j:j + 1])
            ey = jse.tile([P, d], f32, tag="ey")
            nc.scalar.activation(ey[:], yt[:], AF.Exp,
                                 accum_out=sy_sb[:, j:j + 1])
            p2 = jstmp.tile([P, d], f32, tag="jt", name=f"p2_{j}")
            nc.vector.tensor_mul(p2[:], ex[:], xt[:])
            nc.vector.reduce_sum(exs_sb[:, j:j + 1], p2[:], axis=AX.X)
            p3 = jstmp.tile([P, d], f32, tag="jt", name=f"p3_{j}")
            nc.vector.tensor_mul(p3[:], ey[:], yt[:])
            nc.vector.reduce_sum(eys_sb[:, j:j + 1], p3[:], axis=AX.X)
            rsx = tiny.tile([P, 1], f32, tag="rsx")
            nc.vector.reciprocal(rsx[:], sx_sb[:, j:j + 1])
            rsy = tiny.tile([P, 1], f32, tag="rsy")
            nc.vector.reciprocal(rsy[:], sy_sb[:, j:j + 1])
            nc.scalar.activation(ex[:], ex[:], AF.Identity, scale=rsx[:])
            nc.scalar.activation(ey[:], ey[:], AF.Identity, scale=rsy[:])
            tt = jstmp.tile([P, d], f32, tag="jt", name=f"tt_{j}")
            nc.vector.tensor_add(tt[:], ex[:], ey[:])
            lt = jstmp.tile([P, d], f32, tag="jt", name=f"lt_{j}")
            nc.scalar.activation(lt[:], tt[:], AF.Ln, scale=0.5)
            w = jstmp.tile([P, d], f32, tag="jt", name=f"w_{j}")
            nc.vector.tensor_mul(w[:], tt[:], lt[:])
            nc.vector.reduce_sum(w_sb[:, j:j + 1], w[:], axis=AX.X)

        # ---- main loop: S blocks with fused exp + row accumulate ----
        js_every = max(1, nt // max(1, nj))
        jnext = 0
        for t in range(nt):
            ps_tiles = [[mpsum.tile([P, free], f32, tag="mm",
                                    name=f"ps_t{t}_{m}_{g}")
                         for g in range(ng)] for m in range(2)]
            for ch in range(nch):
                lhs = xr_sb[:, ch * rb + t * P: ch * rb + (t + 1) * P]
                for m, src in enumerate((xc_sb, yc_sb)):
                    for g in range(ng):
                        nc.tensor.matmul(
                            ps_tiles[m][g][:], lhs,
                            src[:, ch * cb + g * free: ch * cb + (g + 1) * free],
                            start=(ch == 0), stop=(ch == nch - 1))
            for m in range(2):
                for g in range(ng):
                    scratch = expp.tile([P, free], f32)
                    col = t * ncol + m * ng + g
                    nc.scalar.activation(
                        scratch[:], ps_tiles[m][g][:], AF.Exp,
                        scale=invr_T[:, t:t + 1],
                        accum_out=rs_acc[:, col:col + 1])
            if t % js_every == js_every - 1 and jnext < nj:
                emit_js(jnext)
                jnext += 1
        while jnext < nj:
            emit_js(jnext)
            jnext += 1

        # ---- outputs ----
        for name, sb in (("rs_out", rs_acc), ("ssx_out", ssx_sb),
                         ("ssy_out", ssy_sb), ("dot_out", dot_sb),
                         ("sx_out", sx_sb), ("sy_out", sy_sb),
                         ("exs_out", exs_sb), ("eys_out", eys_sb),
                         ("w_out", w_sb)):
            nc.sync.dma_start(ap(name), sb[:])


def _declare(nc, d=D, rb=RB, cb=CB, jb=JB, free=FREE):
    import concourse.mybir as mybir
    f32 = mybir.dt.float32
    f32r = mybir.dt.float32r
    nt, ng, nj = rb // P, cb // free, jb // P
    io = {}
    for name, shape, kind in (
        ("xrT", [d, rb], "in_f32r"),
        ("xcT", [d, cb], "in_f32r"),
        ("ycT", [d, cb], "in_f32r"),
        ("xrj", [rb, d], "ExternalInput"),
        ("xj", [jb, d], "ExternalInput"),
        ("yj", [jb, d], "ExternalInput"),
        ("rs_out", [P, nt * 2 * ng], "ExternalOutput"),
        ("ssx_out", [P, nt], "ExternalOutput"),
        ("ssy_out", [P, nj], "ExternalOutput"),
        ("dot_out", [P, nj], "ExternalOutput"),
        ("sx_out", [P, nj], "ExternalOutput"),
        ("sy_out", [P, nj], "ExternalOutput"),
        ("exs_out", [P, nj], "ExternalOutput"),
        ("eys_out", [P, nj], "ExternalOutput"),
        ("w_out", [P, nj], "ExternalOutput"),
    ):
        dt = f32r if kind == "in_f32r" else f32
        kind = "ExternalInput" if kind == "in_f32r" else kind
        io[name] = nc.dram_tensor(name, shape, dt, kind=kind).ap()
    return io


def build_nc(d=D, rb=RB, cb=CB, jb=JB, free=FREE, num_devices=NCORES):
    import concourse.tile as tile
    from concourse import bacc
    nc = bacc.Bacc("TRN2", target_bir_lowering=False, debug=False,
                   num_devices=num_devices)
    io = _declare(nc, d, rb, cb, jb, free)
    with tile.TileContext(nc) as tc:
        build(nc, tc, io, d, rb, cb, jb, free)
    nc.compile()
    return nc


def make_in_maps(x, y):
    """Shard full inputs into per-core input maps."""
    x = np.ascontiguousarray(np.asarray(x, dtype=np.float32))
    y = np.ascontiguousarray(np.asarray(y, dtype=np.float32))
    xT = np.ascontiguousarray(x.T)
    yT = np.ascontiguousarray(y.T)
    in_maps = []
    for c in range(NCORES):
        r, g = divmod(c, CGRP)
        rows = slice(r * RB, (r + 1) * RB)
        cols = slice(g * CB, (g + 1) * CB)
        jrows = slice(r * RB + g * JB, r * RB + (g + 1) * JB)
        in_maps.append({
            "xrT": np.ascontiguousarray(xT[:, rows]),
            "xcT": np.ascontiguousarray(xT[:, cols]),
            "ycT": np.ascontiguousarray(yT[:, cols]),
            "xrj": np.ascontiguousarray(x[rows]),
            "xj": np.ascontiguousarray(x[jrows]),
            "yj": np.ascontiguousarray(y[jrows]),
        })
    return in_maps


def combine(results):
    """Combine per-core outputs into the final loss (host O(N) finish)."""
    rs = np.zeros(N)
    sub = np.zeros(N)
    cos_all = np.zeros(N)
    js_sum = 0.0
    ncol = 2 * (CB // FREE)
    for c in range(NCORES):
        r, g = divmod(c, CGRP)
        o = results[c]
        rs_block = o["rs_out"].astype(np.float64).reshape(P, RB // P, ncol).sum(-1)
        rs[r * RB:(r + 1) * RB] += rs_block.T.reshape(RB)
        jrows = slice(r * RB + g * JB, r * RB + (g + 1) * JB)
        nj = JB // P
        ssx_j = o["ssx_out"].astype(np.float64)[:, g * nj:(g + 1) * nj].T.reshape(JB)
        ssy = o["ssy_out"].astype(np.float64).T.reshape(JB)
        dot = o["dot_out"].astype(np.float64).T.reshape(JB)
        cos = dot / np.sqrt(ssx_j * ssy)
        cos_all[jrows] = cos
        sub[jrows] = np.exp(1.0 / T) + np.exp(cos / T)
        sx = o["sx_out"].astype(np.float64)
        sy = o["sy_out"].astype(np.float64)
        js_sum += (o["exs_out"] / sx - np.log(sx)
                   + o["eys_out"] / sy - np.log(sy)
                   - o["w_out"].astype(np.float64)).sum()
    rs -= sub
    neg = np.cumsum(rs)
    nce = np.sum(np.log(neg)) - np.sum(cos_all) / T
    js = 0.5 * js_sum / N
    return np.array([nce + js], dtype=np.float32)


_NC_CACHE = {}


def _get_nc():
    if "nc" not in _NC_CACHE:
        _NC_CACHE["nc"] = build_nc()
    return _NC_CACHE["nc"]


def run(x, y, trace=False, **kw):
    from concourse import bass_utils
    nc = _get_nc()
    in_maps = make_in_maps(x, y)
    res = bass_utils.run_bass_kernel_spmd(
        nc, in_maps, core_ids=list(range(NCORES)), trace=trace, **kw)
    return combine(res.results), res


def kernel(x, y):
    out, _ = run(x, y)
    return out
